# revision 3
# baseline (speedup 1.0000x reference)
"""TRN2 Bass kernel for nn_DQN (topk_masking).

reference:
    h = relu(x @ W1 + b1); h = relu(h @ W2 + b2); logits = h @ W3 + b3
    mask[b, possible_moves[b, :]] = 1
    out = softmax(logits * mask, axis=1)

Strategy (8 NeuronCores, data-parallel over batch, 2048 rows/core):
  - host: transpose x -> xT [128, B]; fold b2/b3 into augmented weight rows.
  - PE: h1T/h2T computed transposed ([hid, batch]) so W1/W2aug are the
    stationary operands; logits via lhsT = h2aug columns (K=25 incl. ones row
    so b3 is free), rhs = W3aug slices; [128, 512] PSUM chunks.
  - GPSIMD local_scatter builds the 0/1 mask [128 rows, 4096] bf16 from
    possible_moves (3 scatters of index ranges [0,2046)/[2046,4092)/[4092,4096)
    since local_scatter caps num_elems at 2046; DVE computes the per-range
    poisoned indices: out-of-range -> negative = ignored by the ucode).
  - DVE: filtered = logits * mask (fp32 x bf16).
  - ACT: E = exp(filtered) with accum_out giving the row sum Z directly
    (illegal positions contribute exp(0)=1 exactly as the reference does).
  - out = E * (1/Z): per-partition tensor_scalar, split DVE/ACT for balance.
"""

import os
import sys

import numpy as np

for _p in ("/root/.axon_site", "/root/.axon_site/_ro/trn_rl_repo",
           "/root/.axon_site/_ro/pypackages"):
    if os.path.isdir(_p) and _p not in sys.path:
        sys.path.append(_p)

B, IN_DIM, HID, OUT_DIM, K = 16384, 128, 24, 4096, 256
NCORES = 8
BS = B // NCORES          # 2048 rows per core
NT = BS // 128            # 16 tiles of 128 rows
HAUG = HID + 1            # 25: hidden + ones row

_cache = {}


def _build_nc():
    import concourse.bacc as bacc
    import concourse.mybir as mybir
    import concourse.tile as tile

    F32 = mybir.dt.float32
    BF16 = mybir.dt.bfloat16
    I32 = mybir.dt.int32
    I16 = mybir.dt.int16
    ALU = mybir.AluOpType
    ACTF = mybir.ActivationFunctionType

    nc = bacc.Bacc("TRN2", target_bir_lowering=False, debug=False,
                   num_devices=NCORES)

    xT = nc.dram_tensor("xT", [IN_DIM, BS], F32, kind="ExternalInput").ap()
    pm = nc.dram_tensor("pm", [BS, K], I32, kind="ExternalInput").ap()
    w1 = nc.dram_tensor("w1", [IN_DIM, HID], F32, kind="ExternalInput").ap()
    b1 = nc.dram_tensor("b1", [HID, 1], F32, kind="ExternalInput").ap()
    w2a = nc.dram_tensor("w2a", [HAUG, HID], F32, kind="ExternalInput").ap()
    w3a = nc.dram_tensor("w3a", [HAUG, OUT_DIM], F32, kind="ExternalInput").ap()
    out = nc.dram_tensor("out", [BS, OUT_DIM], F32, kind="ExternalOutput").ap()

    with tile.TileContext(nc) as tc:
        with tc.tile_pool(name="singles", bufs=1) as singles:
            xT_s = singles.tile([IN_DIM, BS], F32)
            nc.sync.dma_start(out=xT_s, in_=xT)
            w1_s = singles.tile([IN_DIM, HID], F32)
            nc.sync.dma_start(out=w1_s, in_=w1)
            b1_s = singles.tile([HID, 1], F32)
            nc.sync.dma_start(out=b1_s, in_=b1)
            w2a_s = singles.tile([HAUG, HID], F32)
            nc.sync.dma_start(out=w2a_s, in_=w2a)
            w3a_s = singles.tile([HAUG, OUT_DIM], F32)
            nc.sync.dma_start(out=w3a_s, in_=w3a)
            ones_s = singles.tile([128, K], BF16)
            nc.vector.memset(ones_s, 1.0)
            # ones row (partition 24) can't be memset alone: engine base
            # partition must be 0/32/64/96 -> memset whole tile, relu
            # overwrites rows 0..23
            h2a_s = singles.tile([HAUG, BS], F32)
            nc.vector.memset(h2a_s, 1.0)

            # ---- tiny MLP: h2aug [25, BS], computed in 512-col chunks ----
            with tc.tile_pool(name="mlp_ps", bufs=2, space="PSUM") as mlp_ps, \
                 tc.tile_pool(name="mlp", bufs=2) as mlp:
                for c in range(BS // 512):
                    sl = slice(c * 512, (c + 1) * 512)
                    p1 = mlp_ps.tile([HID, 512], F32, tag="p1")
                    nc.tensor.matmul(p1, w1_s, xT_s[:, sl], start=True,
                                     stop=True)
                    h1a = mlp.tile([HAUG, 512], F32, tag="h1")
                    nc.vector.memset(h1a, 1.0)
                    nc.scalar.activation(h1a[0:HID, :], p1, ACTF.Relu,
                                         bias=b1_s)
                    p2 = mlp_ps.tile([HID, 512], F32, tag="p2")
                    nc.tensor.matmul(p2, w2a_s, h1a, start=True, stop=True)
                    nc.scalar.activation(h2a_s[0:HID, sl], p2, ACTF.Relu)

            # ---- main loop over 16 tiles of 128 batch rows ----
            with tc.tile_pool(name="io", bufs=3) as iop, \
                 tc.tile_pool(name="idx", bufs=3) as idxp, \
                 tc.tile_pool(name="mask", bufs=2) as maskp, \
                 tc.tile_pool(name="big", bufs=3) as bigp, \
                 tc.tile_pool(name="epool", bufs=4) as epool, \
                 tc.tile_pool(name="outp", bufs=2) as outp, \
                 tc.tile_pool(name="ps", bufs=2, space="PSUM") as psp, \
                 tc.tile_pool(name="small", bufs=4) as smallp:
                for t in range(NT):
                    rows = slice(t * 128, (t + 1) * 128)

                    pm_s = iop.tile([128, K], I32, tag="pm")
                    nc.scalar.dma_start(out=pm_s, in_=pm[rows, :])

                    # poisoned per-range indices; negatives are ignored by
                    # the scatter ucode, so only too-high values need help
                    v0 = idxp.tile([128, K], I16, tag="v0")
                    nc.vector.tensor_scalar(v0, pm_s, 2046, None, ALU.is_lt)
                    idx0 = idxp.tile([128, K], I16, tag="i0")
                    nc.vector.affine_then_add(idx0, v0, pm_s, 4096.0, -4096.0)
                    v1 = idxp.tile([128, K], I16, tag="v1")
                    nc.vector.tensor_scalar(v1, pm_s, 4092, None, ALU.is_lt)
                    idx1 = idxp.tile([128, K], I16, tag="i1")
                    nc.vector.affine_then_add(idx1, v1, pm_s, 2050.0, -4096.0)
                    idx2 = idxp.tile([128, K], I16, tag="i2")
                    nc.vector.tensor_scalar(idx2, pm_s, 4092, None,
                                            ALU.subtract)

                    m = maskp.tile([128, OUT_DIM], BF16, tag="m")
                    nc.gpsimd.local_scatter(m[:, 0:2046], ones_s, idx0,
                                            128, 2046, K)
                    nc.gpsimd.local_scatter(m[:, 2046:4092], ones_s, idx1,
                                            128, 2046, K)
                    nc.gpsimd.local_scatter(m[:, 4092:4096], ones_s, idx2,
                                            128, 4, K)

                    out_t = outp.tile([128, OUT_DIM], F32, tag="out")
                    e_tiles = []
                    z_tiles = []
                    for h in range(2):
                        hsl = slice(h * 2048, (h + 1) * 2048)
                        pl = psp.tile([128, 2048], F32, tag="pl")
                        for n in range(4):
                            ns = h * 2048 + n * 512
                            nc.tensor.matmul(
                                pl[:, n * 512:(n + 1) * 512],
                                h2a_s[:, rows],
                                w3a_s[:, ns:ns + 512],
                                start=True, stop=True,
                            )
                        filt = bigp.tile([128, 2048], F32, tag="filt")
                        nc.vector.tensor_tensor(
                            out=filt, in0=pl, in1=m[:, hsl], op=ALU.mult)
                        e = epool.tile([128, 2048], F32, tag="e")
                        zp = smallp.tile([128, 1], F32, tag=f"zp{h}")
                        nc.scalar.activation(e, filt, ACTF.Exp, accum_out=zp)
                        e_tiles.append(e)
                        z_tiles.append(zp)

                    z = smallp.tile([128, 1], F32, tag="z")
                    nc.vector.tensor_tensor(out=z, in0=z_tiles[0],
                                            in1=z_tiles[1], op=ALU.add)
                    invz = smallp.tile([128, 1], F32, tag="invz")
                    nc.vector.reciprocal(invz, z)

                    # normalize: half on DVE (2x fp32), half on ACT
                    nc.vector.tensor_scalar(out_t[:, 0:2048], e_tiles[0],
                                            invz, None, ALU.mult)
                    nc.scalar.mul(out_t[:, 2048:4096], e_tiles[1], invz)

                    nc.sync.dma_start(out=out[rows, :], in_=out_t)

    nc.compile()
    return nc


def _get_nc():
    if "nc" not in _cache:
        _cache["nc"] = _build_nc()
    return _cache["nc"]


def _prep_inputs(x, possible_moves, W1, b1, W2, b2, W3, b3):
    x = np.ascontiguousarray(np.asarray(x, dtype=np.float32))
    pm = np.ascontiguousarray(np.asarray(possible_moves).astype(np.int32))
    W1 = np.ascontiguousarray(np.asarray(W1, dtype=np.float32))
    b1c = np.asarray(b1, dtype=np.float32).reshape(HID, 1)
    w2a = np.ascontiguousarray(
        np.concatenate([np.asarray(W2, np.float32),
                        np.asarray(b2, np.float32)[None, :]], axis=0))
    w3a = np.ascontiguousarray(
        np.concatenate([np.asarray(W3, np.float32),
                        np.asarray(b3, np.float32)[None, :]], axis=0))
    xT = np.ascontiguousarray(x.T)  # [IN_DIM, B]

    in_maps = []
    for c in range(NCORES):
        sl = slice(c * BS, (c + 1) * BS)
        in_maps.append({
            "xT": np.ascontiguousarray(xT[:, sl]),
            "pm": np.ascontiguousarray(pm[sl, :]),
            "w1": W1,
            "b1": b1c,
            "w2a": w2a,
            "w3a": w3a,
        })
    return in_maps


def kernel(x, possible_moves, W1, b1, W2, b2, W3, b3):
    from concourse.bass_utils import run_bass_kernel_spmd

    in_maps = _prep_inputs(x, possible_moves, W1, b1, W2, b2, W3, b3)
    nc = _get_nc()
    res = run_bass_kernel_spmd(nc, in_maps, core_ids=list(range(NCORES)))
    return np.concatenate([res.results[c]["out"] for c in range(NCORES)],
                          axis=0)


# revision 6
# speedup vs baseline: 10.1708x; 10.1708x over previous
"""TRN2 Bass kernel for nn_DQN (topk_masking).

reference:
    h = relu(x @ W1 + b1); h = relu(h @ W2 + b2); logits = h @ W3 + b3
    mask[b, possible_moves[b, :]] = 1
    out = softmax(logits * mask, axis=1)

Strategy (8 NeuronCores, data-parallel over batch, 2048 rows/core):
  - host: transpose x -> xT [128, B]; fold b2/b3 into augmented weight rows.
  - PE: h1T/h2T computed transposed ([hid, batch]) so W1/W2aug are the
    stationary operands; logits via lhsT = h2aug columns (K=25 incl. ones row
    so b3 is free), rhs = W3aug slices; [128, 512] PSUM chunks.
  - GPSIMD local_scatter builds the 0/1 mask [128 rows, 4096] bf16 from
    possible_moves (3 scatters of index ranges [0,2046)/[2046,4092)/[4092,4096)
    since local_scatter caps num_elems at 2046; DVE computes the per-range
    poisoned indices: out-of-range -> negative = ignored by the ucode).
  - DVE: filtered = logits * mask (fp32 x bf16).
  - ACT: E = exp(filtered) with accum_out giving the row sum Z directly
    (illegal positions contribute exp(0)=1 exactly as the reference does).
  - out = E * (1/Z): per-partition tensor_scalar, split DVE/ACT for balance.
"""

import os
import sys

import numpy as np

for _p in ("/root/.axon_site", "/root/.axon_site/_ro/trn_rl_repo",
           "/root/.axon_site/_ro/pypackages"):
    if os.path.isdir(_p) and _p not in sys.path:
        sys.path.append(_p)

B, IN_DIM, HID, OUT_DIM, K = 16384, 128, 24, 4096, 256
NCORES = 8
BS = B // NCORES          # 2048 rows per core
NT = BS // 128            # 16 tiles of 128 rows
HAUG = HID + 1            # 25: hidden + ones row

_cache = {}


def _build_nc(reps=1):
    import concourse.bacc as bacc
    import concourse.mybir as mybir
    import concourse.tile as tile

    F32 = mybir.dt.float32
    BF16 = mybir.dt.bfloat16
    I32 = mybir.dt.int32
    I16 = mybir.dt.int16
    ALU = mybir.AluOpType
    ACTF = mybir.ActivationFunctionType

    nc = bacc.Bacc("TRN2", target_bir_lowering=False, debug=False,
                   num_devices=NCORES)

    xT = nc.dram_tensor("xT", [IN_DIM, BS], F32, kind="ExternalInput").ap()
    pm = nc.dram_tensor("pm", [BS, K], I32, kind="ExternalInput").ap()
    w1 = nc.dram_tensor("w1", [IN_DIM, HID], F32, kind="ExternalInput").ap()
    b1 = nc.dram_tensor("b1", [HID, 1], F32, kind="ExternalInput").ap()
    w2a = nc.dram_tensor("w2a", [HAUG, HID], F32, kind="ExternalInput").ap()
    w3a = nc.dram_tensor("w3a", [HAUG, OUT_DIM], F32, kind="ExternalInput").ap()
    out = nc.dram_tensor("out", [BS, OUT_DIM], F32, kind="ExternalOutput").ap()

    with tile.TileContext(nc) as tc:
        with tc.tile_pool(name="singles", bufs=1) as singles:
            xT_s = singles.tile([IN_DIM, BS], F32)
            nc.sync.dma_start(out=xT_s, in_=xT)
            w1_s = singles.tile([IN_DIM, HID], F32)
            nc.sync.dma_start(out=w1_s, in_=w1)
            b1_s = singles.tile([HID, 1], F32)
            nc.sync.dma_start(out=b1_s, in_=b1)
            w2a_s = singles.tile([HAUG, HID], F32)
            nc.sync.dma_start(out=w2a_s, in_=w2a)
            w3a_s = singles.tile([HAUG, OUT_DIM], F32)
            nc.sync.dma_start(out=w3a_s, in_=w3a)
            ones_s = singles.tile([128, K], BF16)
            nc.vector.memset(ones_s, 1.0)
            # ones row (partition 24) can't be memset alone: engine base
            # partition must be 0/32/64/96 -> memset whole tile, relu
            # overwrites rows 0..23
            h2a_s = singles.tile([HAUG, BS], F32)
            nc.vector.memset(h2a_s, 1.0)

            # ---- tiny MLP: h2aug [25, BS], computed in 512-col chunks ----
            with tc.tile_pool(name="mlp_ps", bufs=2, space="PSUM") as mlp_ps, \
                 tc.tile_pool(name="mlp", bufs=2) as mlp:
                for c in range(BS // 512):
                    sl = slice(c * 512, (c + 1) * 512)
                    p1 = mlp_ps.tile([HID, 512], F32, tag="p1")
                    nc.tensor.matmul(p1, w1_s, xT_s[:, sl], start=True,
                                     stop=True)
                    h1a = mlp.tile([HAUG, 512], F32, tag="h1")
                    nc.vector.memset(h1a, 1.0)
                    nc.scalar.activation(h1a[0:HID, :], p1, ACTF.Relu,
                                         bias=b1_s)
                    p2 = mlp_ps.tile([HID, 512], F32, tag="p2")
                    nc.tensor.matmul(p2, w2a_s, h1a, start=True, stop=True)
                    nc.scalar.activation(h2a_s[0:HID, sl], p2, ACTF.Relu)

            # ---- main loop over 16 tiles of 128 batch rows ----
            with tc.tile_pool(name="io", bufs=3) as iop, \
                 tc.tile_pool(name="idx", bufs=3) as idxp, \
                 tc.tile_pool(name="mask", bufs=2) as maskp, \
                 tc.tile_pool(name="big", bufs=3) as bigp, \
                 tc.tile_pool(name="epool", bufs=4) as epool, \
                 tc.tile_pool(name="outp", bufs=2) as outp, \
                 tc.tile_pool(name="ps", bufs=2, space="PSUM") as psp, \
                 tc.tile_pool(name="small", bufs=4) as smallp:
                for t in [i % NT for i in range(NT * reps)]:
                    rows = slice(t * 128, (t + 1) * 128)

                    pm_s = iop.tile([128, K], I32, tag="pm")
                    nc.scalar.dma_start(out=pm_s, in_=pm[rows, :])

                    # poisoned per-range indices; negatives are ignored by
                    # the scatter ucode, so only too-high values need help
                    v0 = idxp.tile([128, K], I16, tag="v0")
                    nc.vector.tensor_scalar(v0, pm_s, 2046, None, ALU.is_lt)
                    idx0 = idxp.tile([128, K], I16, tag="i0")
                    nc.vector.affine_then_add(idx0, v0, pm_s, 4096.0, -4096.0)
                    v1 = idxp.tile([128, K], I16, tag="v1")
                    nc.vector.tensor_scalar(v1, pm_s, 4092, None, ALU.is_lt)
                    idx1 = idxp.tile([128, K], I16, tag="i1")
                    nc.vector.affine_then_add(idx1, v1, pm_s, 2050.0, -4096.0)
                    idx2 = idxp.tile([128, K], I16, tag="i2")
                    nc.vector.tensor_scalar(idx2, pm_s, 4092, None,
                                            ALU.subtract)

                    m = maskp.tile([128, OUT_DIM], BF16, tag="m")
                    nc.gpsimd.local_scatter(m[:, 0:2046], ones_s, idx0,
                                            128, 2046, K)
                    nc.gpsimd.local_scatter(m[:, 2046:4092], ones_s, idx1,
                                            128, 2046, K)
                    nc.gpsimd.local_scatter(m[:, 4092:4096], ones_s, idx2,
                                            128, 4, K)

                    out_t = outp.tile([128, OUT_DIM], F32, tag="out")
                    e_tiles = []
                    z_tiles = []
                    for h in range(2):
                        hsl = slice(h * 2048, (h + 1) * 2048)
                        pl = psp.tile([128, 2048], F32, tag="pl")
                        for n in range(4):
                            ns = h * 2048 + n * 512
                            nc.tensor.matmul(
                                pl[:, n * 512:(n + 1) * 512],
                                h2a_s[:, rows],
                                w3a_s[:, ns:ns + 512],
                                start=True, stop=True,
                            )
                        filt = bigp.tile([128, 2048], F32, tag="filt")
                        nc.vector.tensor_tensor(
                            out=filt, in0=pl, in1=m[:, hsl], op=ALU.mult)
                        e = epool.tile([128, 2048], F32, tag="e")
                        zp = smallp.tile([128, 1], F32, tag=f"zp{h}")
                        nc.scalar.activation(e, filt, ACTF.Exp, accum_out=zp)
                        e_tiles.append(e)
                        z_tiles.append(zp)

                    z = smallp.tile([128, 1], F32, tag="z")
                    nc.vector.tensor_tensor(out=z, in0=z_tiles[0],
                                            in1=z_tiles[1], op=ALU.add)
                    invz = smallp.tile([128, 1], F32, tag="invz")
                    nc.vector.reciprocal(invz, z)

                    # normalize: half on DVE (2x fp32), half on ACT
                    nc.vector.tensor_scalar(out_t[:, 0:2048], e_tiles[0],
                                            invz, None, ALU.mult)
                    nc.scalar.mul(out_t[:, 2048:4096], e_tiles[1], invz)

                    nc.sync.dma_start(out=out[rows, :], in_=out_t)

    nc.compile()
    return nc


def _get_nc(reps=1):
    key = f"nc{reps}"
    if key not in _cache:
        _cache[key] = _build_nc(reps)
    return _cache[key]


def _prep_inputs(x, possible_moves, W1, b1, W2, b2, W3, b3):
    x = np.ascontiguousarray(np.asarray(x, dtype=np.float32))
    pm = np.ascontiguousarray(np.asarray(possible_moves).astype(np.int32))
    W1 = np.ascontiguousarray(np.asarray(W1, dtype=np.float32))
    b1c = np.asarray(b1, dtype=np.float32).reshape(HID, 1)
    w2a = np.ascontiguousarray(
        np.concatenate([np.asarray(W2, np.float32),
                        np.asarray(b2, np.float32)[None, :]], axis=0))
    w3a = np.ascontiguousarray(
        np.concatenate([np.asarray(W3, np.float32),
                        np.asarray(b3, np.float32)[None, :]], axis=0))
    xT = np.ascontiguousarray(x.T)  # [IN_DIM, B]

    in_maps = []
    for c in range(NCORES):
        sl = slice(c * BS, (c + 1) * BS)
        in_maps.append({
            "xT": np.ascontiguousarray(xT[:, sl]),
            "pm": np.ascontiguousarray(pm[sl, :]),
            "w1": W1,
            "b1": b1c,
            "w2a": w2a,
            "w3a": w3a,
        })
    return in_maps


def kernel(x, possible_moves, W1, b1, W2, b2, W3, b3):
    from concourse.bass_utils import run_bass_kernel_spmd

    in_maps = _prep_inputs(x, possible_moves, W1, b1, W2, b2, W3, b3)
    nc = _get_nc()
    res = run_bass_kernel_spmd(nc, in_maps, core_ids=list(range(NCORES)))
    return np.concatenate([res.results[c]["out"] for c in range(NCORES)],
                          axis=0)
